# revision 2
# baseline (speedup 1.0000x reference)
import os
import sys

sys.path.insert(0, "/root/problem")
sys.path.insert(0, "/opt/trn_rl_repo")

import numpy as np

import tilefix  # noqa: F401  (walrus single-sync-wait workaround)
import conv_kernel

_NC_CACHE = {}
LAST_EXEC_NS = None


def _get_nc():
    if "nc" not in _NC_CACHE:
        _NC_CACHE["nc"] = conv_kernel.build()
    return _NC_CACHE["nc"]


def _binarize(w):
    return np.where(w >= 0, 1.0, -1.0).astype(np.float16)


def _prep_weights(w1, b1, w2, b2, w3, b3, w4, b4):
    w1T = _binarize(w1).reshape(128, 9).T  # [9, 128]
    w1sb = np.zeros((128, 128), np.float16)
    for g in range(4):
        w1sb[32 * g:32 * g + 9, :] = w1T

    wb2 = _binarize(w2)  # [64, 128, 3, 3]
    w2sb = np.zeros((128, 576), np.float16)
    for t in range(9):
        dy, dx = divmod(t, 3)
        w2sb[:, 64 * t:64 * t + 64] = wb2[:, :, dy, dx].T

    wb3 = _binarize(w3)  # [32, 64, 3, 3]
    w3sb = np.zeros((64, 288), np.float16)
    for t in range(9):
        dy, dx = divmod(t, 3)
        w3sb[:, 32 * t:32 * t + 32] = wb3[:, :, dy, dx].T

    wb4 = _binarize(w4)  # [8, 32, 3, 2]
    w4sb = np.zeros((32, 48), np.float16)
    for t in range(6):
        dy, dx = divmod(t, 2)
        w4sb[:, 8 * t:8 * t + 8] = wb4[:, :, dy, dx].T

    return {
        "w1sb": w1sb, "w2sb": w2sb, "w3sb": w3sb, "w4sb": w4sb,
        "b1": np.asarray(b1, np.float32).reshape(128, 1),
        "b2": np.asarray(b2, np.float32).reshape(64, 1),
        "b3": np.asarray(b3, np.float32).reshape(32, 1),
        "b4": np.asarray(b4, np.float32).reshape(8, 1),
    }


def kernel(x, w1, b1, w2, b2, w3, b3, w4, b4):
    global LAST_EXEC_NS
    from concourse.bass_utils import run_bass_kernel_spmd

    nc = _get_nc()
    wmap = _prep_weights(w1, b1, w2, b2, w3, b3, w4, b4)

    x = np.asarray(x, np.float32)  # [32, 1, 224, 224]
    in_maps = []
    for c in range(8):
        shard = x[4 * c:4 * c + 4, 0].astype(np.float16).ravel()
        m = dict(wmap)
        m["xh"] = np.concatenate([shard, np.zeros(64, np.float16)])
        in_maps.append(m)

    trace = bool(int(os.environ.get("CONV_TRACE", "0")))
    res = run_bass_kernel_spmd(nc, in_maps, core_ids=list(range(8)), trace=trace)
    LAST_EXEC_NS = res.exec_time_ns
    out = np.concatenate([res.results[c]["out"] for c in range(8)], axis=0)
    return out.astype(np.float32)
